# revision 49
# baseline (speedup 1.0000x reference)
"""LoRA-XS Linear fused kernel for 8 TRN2 NeuronCores.

out[b,s,o] = x @ (W + U @ sigma @ R @ Vt)^T + bias

Strategy:
  - Host: fold the rank-64 LoRA delta into W (tiny: ~0.5 GFLOP), round
    x / W_eff to fp32r (e8m11, bit-exact with the PE's own rounding),
    and lay out operands k-major for the tensor engine.
  - Device: 8-way data-parallel over the 8192 rows; each core computes
    a 1024x2048 @ 2048x2048 matmul with fp32r 1-pass matmuls (1 cyc/row
    at FD=512, 4x the native fp32 rate) accumulating in fp32 PSUM, plus
    a fused bias add on PSUM eviction.

Shapes (hardcoded): x (4, 2048, 2048) f32, weight (2048, 2048) f32,
bias (2048,) f32, U (2048, 64), sigma/R (64, 64), Vt (64, 2048).
"""

import sys

sys.path.insert(0, "/opt/trn_rl_repo")

import numpy as np

import concourse.bass as bass
import concourse.bacc as bacc
import concourse.mybir as mybir
import concourse.tile as tile
from concourse.bass_utils import run_bass_kernel_spmd

F32 = mybir.dt.float32
F32R = mybir.dt.float32r

ALPHA = 1.0
NCORES = 8
P = 128
B, S, D_IN, D_OUT = 4, 2048, 2048, 2048
ROWS = B * S  # 8192
ROWS_PER_CORE = ROWS // NCORES  # 1024
MT = ROWS_PER_CORE // P  # 8 m-tiles per core
KT = D_IN // P  # 16 k-tiles
NFD = 512  # matmul free dim (one PSUM bank of fp32)
NT = D_OUT // NFD  # 4 n-tiles

_CACHE = {}


def _round_fp32r(a: np.ndarray) -> np.ndarray:
    """RNE-round fp32 to the PE's fp32r (e8m11) — matches walrus
    fp32_to_fp32r bit-for-bit (probe-verified on hardware)."""
    u = np.ascontiguousarray(a).view(np.uint32)
    r = (u + np.uint32(0x7FF) + ((u >> np.uint32(12)) & np.uint32(1))) & np.uint32(
        0xFFFFF000
    )
    return r.view(np.float32)


def _build():
    nc = bacc.Bacc(None, target_bir_lowering=False, debug=False)
    xt = nc.dram_tensor("xt", [P, MT, KT, P], F32R, kind="ExternalInput").ap()
    wt = nc.dram_tensor("wt", [P, KT, D_OUT], F32R, kind="ExternalInput").ap()
    bias = nc.dram_tensor("bias", [D_OUT], F32, kind="ExternalInput").ap()
    out = nc.dram_tensor("out", [P, MT, D_OUT], F32, kind="ExternalOutput").ap()

    with tile.TileContext(nc) as tc:
        with (
            tc.tile_pool(name="const", bufs=1) as const,
            tc.tile_pool(name="xpool", bufs=MT) as xpool,
            tc.tile_pool(name="wpool", bufs=8) as wpool,
            tc.tile_pool(name="opool", bufs=32) as opool,
            tc.tile_pool(name="psum", bufs=MT, space="PSUM") as psum,
        ):
            # k-quarter burst schedule: every (quarter, m-tile, k-quarter)
            # is a 4-matmul PSUM burst evicted immediately into an SBUF
            # accumulator, so PSUM banks recycle in ~1us and the q0/q1
            # work can interleave during the x ingest — the PE work
            # enabled per streamed MB exceeds the DMA rate from the
            # start, instead of being gated by whole-phase accumulations.
            x_tiles = [
                xpool.tile([P, KT, P], F32R, name=f"x_{mm}", tag="x")
                for mm in range(MT)
            ]
            w_tiles = {}

            def load_w(q, kc):
                # one DMA covers two adjacent k-slices of this d_out quarter
                t = wpool.tile([P, 2, NFD], F32R, name=f"w_{q}_{kc}", tag="w")
                nc.sync.dma_start(
                    out=t[:], in_=wt[:, kc : kc + 2, q * NFD : (q + 1) * NFD]
                )
                w_tiles[(q, kc + 1)] = t[:, 1, :]
                w_tiles[(q, kc)] = t[:, 0, :]

            # bias first: 8KB DMA + replicate on the idle GpSimd engine
            # (needed by the very first burst eviction at ~5us)
            bias_sb = const.tile([1, D_OUT], F32)
            bias_bc = const.tile([P, D_OUT], F32)
            bias_ap = bass.AP(
                tensor=bias.tensor,
                offset=bias.offset,
                ap=[[0, 1], [1, D_OUT]],
            )
            nc.sync.dma_start(out=bias_sb[:], in_=bias_ap)
            nc.gpsimd.partition_broadcast(bias_bc[:], bias_sb[:])

            # DMA stream in consumption order: per k-quarter step the x
            # chunks plus the q0 AND q1 W pairs (both quarters' bursts run
            # during the ingest), then the q2/q3 W stream.
            for kq in range(4):
                nc.sync.dma_start(
                    out=x_tiles[0][:, 4 * kq : 4 * kq + 4, :],
                    in_=xt[:, 0, 4 * kq : 4 * kq + 4, :],
                )
                load_w(0, 4 * kq)
                load_w(0, 4 * kq + 2)
                for mm in range(1, MT):
                    nc.sync.dma_start(
                        out=x_tiles[mm][:, 4 * kq : 4 * kq + 4, :],
                        in_=xt[:, mm, 4 * kq : 4 * kq + 4, :],
                    )
                load_w(1, 4 * kq)
                load_w(1, 4 * kq + 2)
                load_w(2, 4 * kq)
                load_w(2, 4 * kq + 2)
            for kq in range(4):
                load_w(3, 4 * kq)
                load_w(3, 4 * kq + 2)

            # zero scratch for PE warm-up matmuls (fills the initial DMA
            # gate; the first real start=True matmul clears the bank)
            scratch = const.tile([P, NFD], F32)
            nc.vector.memset(scratch[:], 0.0)

            o_accs = {}

            def burst(q, mm, kq, first=False):
                ps = psum.tile(
                    [P, NFD], F32, name=f"ps_{q}_{mm}_{kq}", tag="acc"
                )
                if first:
                    for _ in range(2):
                        nc.tensor.matmul(
                            ps[:], scratch[:, :P], scratch[:],
                            start=True, stop=True, skip_group_check=True,
                        )
                for i in range(4):
                    kk = 4 * kq + i
                    nc.tensor.matmul(
                        ps[:],
                        x_tiles[mm][:, kk, :],
                        w_tiles[(q, kk)],
                        start=(i == 0),
                        stop=(i == 3),
                    )
                if kq == 0:
                    o = opool.tile(
                        [P, NFD], F32, name=f"o_{q}_{mm}", tag="o"
                    )
                    o_accs[(q, mm)] = o
                    nc.vector.tensor_add(
                        o[:], ps[:], bias_bc[:, q * NFD : (q + 1) * NFD]
                    )
                else:
                    o = o_accs[(q, mm)]
                    nc.vector.tensor_add(o[:], o[:], ps[:])
                if kq == 3:
                    nc.sync.dma_start(
                        out=out[:, mm, q * NFD : (q + 1) * NFD], in_=o[:]
                    )

            for kq in range(4):
                for q in (0, 1, 2):
                    for mm in range(MT):
                        burst(q, mm, kq, first=(q == 0 and mm == 0 and kq == 0))
            for kq in range(4):
                for mm in range(MT):
                    burst(3, mm, kq)

    nc.compile()
    return nc


def _prepare(x, weight, bias, U, sigma, R, Vt):
    """Host prep: fold LoRA delta, fp32r-round, k-major layouts per core."""
    x = np.asarray(x, dtype=np.float32)
    weight = np.asarray(weight, dtype=np.float32)
    bias = np.asarray(bias, dtype=np.float32)
    U = np.asarray(U, dtype=np.float32)
    sigma = np.asarray(sigma, dtype=np.float32)
    R = np.asarray(R, dtype=np.float32)
    Vt = np.asarray(Vt, dtype=np.float32)

    # Fold LoRA delta into the weight (rank-64: negligible host cost)
    w_eff = weight + ALPHA * ((U @ (sigma @ R)) @ Vt)

    # wt[p, kk, n] = w_eff[n, kk*P + p]
    wt = np.ascontiguousarray(
        _round_fp32r(w_eff).T.reshape(KT, P, D_OUT).transpose(1, 0, 2)
    )
    # xt_c[p, mm, kk, j] = x_core[mm*P + j, kk*P + p]
    xr = _round_fp32r(x.reshape(ROWS, D_IN))
    in_maps = []
    for c in range(NCORES):
        shard = xr[c * ROWS_PER_CORE : (c + 1) * ROWS_PER_CORE]
        xt_c = np.ascontiguousarray(
            shard.reshape(MT, P, KT, P).transpose(3, 0, 2, 1)
        )
        in_maps.append({"xt": xt_c, "wt": wt, "bias": bias})
    return in_maps


def _get_nc():
    if "nc" not in _CACHE:
        _CACHE["nc"] = _build()
    return _CACHE["nc"]


def _gather(core_outs):
    # out_full[c*1024 + mm*128 + p, n] = core_outs[c][p, mm, n]
    stacked = np.stack(core_outs)
    full = stacked.transpose(0, 2, 1, 3).reshape(ROWS, D_OUT)
    return full.reshape(B, S, D_OUT)


def kernel(x, weight, bias, U, sigma, R, Vt):
    in_maps = _prepare(x, weight, bias, U, sigma, R, Vt)
    nc = _get_nc()
    res = run_bass_kernel_spmd(nc, in_maps, list(range(NCORES)))
    return _gather([res.results[c]["out"] for c in range(NCORES)])
